# revision 1
# baseline (speedup 1.0000x reference)
"""Trainium2 Bass kernel for the BaseMemory coref scoring module.

Computes, for full inputs (M=65536 memory slots, D=768, E=20, H=64):
    score = relu(pair @ W1 + b1) @ W2 + b2, masked with ent_counter>0,
    where pair = [mem, ment, mem*ment, dist_emb, cnt_emb].

Sharding: data-parallel over the cluster dimension M across 8 NeuronCores.
Each core's shard of mem_vectors is laid out [D, MS] (contraction-major) so
the PE consumes it directly; all FLOPs and all HBM traffic stay on device.

Key algebraic folds (host side, O(D*H) work):
  - mem@W1_mem + (mem*ment)@W1_had = mem @ (W1_mem + diag(ment)@W1_had)
  - ment@W1_ment + b1 folded into the 10-row dist bucket table
  - bucket embedding lookups become one-hot rows contracted on the PE
  - masking folded into the PE accumulation (exact)
"""

import os
import numpy as np

# The bass kernel executes through the axon PJRT backend; make sure jax can
# see it even if the caller pinned JAX_PLATFORMS (e.g. to "cpu").
_jp = os.environ.get("JAX_PLATFORMS")
if _jp is not None and _jp != "" and "axon" not in _jp:
    os.environ["JAX_PLATFORMS"] = "axon," + _jp

M, D, E, H = 65536, 768, 20, 64
N_CORES = 8
MS = M // N_CORES          # rows per core = 8192
GROUP = 512                # rows per PE matmul group
N_GROUPS = MS // GROUP     # 16
SG = 4                     # groups per DMA super-group
N_SG = N_GROUPS // SG      # 4
KCH = D // 128             # 6 contraction chunks
NF = 22                    # 10 dist onehot, 10 cnt onehot, notmask, ones
N_BLK = MS // 128          # 64 feature blocks per core
BIG = float(2 ** 20)       # pre-relu kill value for masked rows

_CACHE = {}


def _build():
    """Build + compile the 8-core SPMD bass program once per process."""
    if "nc" in _CACHE:
        return _CACHE["nc"]

    import concourse.bass as bass
    import concourse.mybir as mybir
    import concourse.tile as tile
    from concourse import bacc
    from concourse.masks import make_identity

    F32 = mybir.dt.float32
    F32R = mybir.dt.float32r

    nc = bacc.Bacc("TRN2", target_bir_lowering=False, debug=False,
                   enable_asserts=False, num_devices=N_CORES)

    xt_d = nc.dram_tensor("xt", [D, MS], F32R, kind="ExternalInput").ap()
    lms_d = nc.dram_tensor("lms", [128, N_BLK], F32, kind="ExternalInput").ap()
    cnt_d = nc.dram_tensor("cnt", [128, N_BLK], F32, kind="ExternalInput").ap()
    w1_d = nc.dram_tensor("w1", [D, H], F32R, kind="ExternalInput").ap()
    tcat_d = nc.dram_tensor("tcat", [NF, H], F32R, kind="ExternalInput").ap()
    wsc_d = nc.dram_tensor("wsc", [H + NF, 1], F32R, kind="ExternalInput").ap()
    lo_d = nc.dram_tensor("lo", [128, NF], F32, kind="ExternalInput").ap()
    hi_d = nc.dram_tensor("hi", [128, NF], F32, kind="ExternalInput").ap()
    out_d = nc.dram_tensor("out", [MS], F32, kind="ExternalOutput").ap()

    # xt[d, m]: tile (k, s) = [128, SG*GROUP] at rows 128k, cols 2048s
    xt_r = xt_d.rearrange("(kp k2 p) (s c) -> p kp k2 s c", p=128, k2=2,
                          s=N_SG)
    w1_r = w1_d.rearrange("(k p) n -> p k n", p=128)    # [128, 6, 64]
    out_r = out_d.rearrange("(s c) -> s c", s=N_SG)  # [4, 2048]

    ge = mybir.AluOpType.is_ge
    le = mybir.AluOpType.is_le
    relu = mybir.ActivationFunctionType.Relu

    with tile.TileContext(nc) as tc:
        with (
            tc.tile_pool(name="consts", bufs=1) as cpool,
            tc.tile_pool(name="feat", bufs=1) as fpool,
            tc.tile_pool(name="xin", bufs=8) as px,
            tc.tile_pool(name="ht", bufs=6) as pht,
            tc.tile_pool(name="osb", bufs=2) as posb,
            tc.tile_pool(name="psf", bufs=2, space="PSUM") as psf,
            tc.tile_pool(name="psz", bufs=4, space="PSUM") as psz,
            tc.tile_pool(name="pss", bufs=2, space="PSUM") as pss,
        ):
            # consts issue on the scalar HWDGE queue so the big xt DMAs
            # (sync queue) start immediately
            ident_t = cpool.tile([128, 128], F32, tag="ident")
            make_identity(nc, ident_t[:])
            ident_r = cpool.tile([128, 128], F32R, tag="identr")
            nc.vector.tensor_copy(ident_r[:], ident_t[:])
            ident = ident_r[:]

            w1t = cpool.tile([128, KCH, H], F32R, tag="w1t")
            nc.scalar.dma_start(w1t[:], w1_r[:])
            lo_t = cpool.tile([128, NF], F32, tag="lo")
            nc.scalar.dma_start(lo_t[:], lo_d[:])
            hi_t = cpool.tile([128, NF], F32, tag="hi")
            nc.scalar.dma_start(hi_t[:], hi_d[:])
            lms_t = cpool.tile([128, N_BLK], F32, tag="lms")
            nc.scalar.dma_start(lms_t[:], lms_d[:])
            cnt_t = cpool.tile([128, N_BLK], F32, tag="cnt")
            nc.scalar.dma_start(cnt_t[:], cnt_d[:])
            tcat_full = cpool.tile([H + NF, H], F32R, tag="tcat")
            tcat = tcat_full[H:H + NF, :]
            nc.scalar.dma_start(tcat, tcat_d[:])
            wsc = cpool.tile([H + NF, 1], F32R, tag="wsc")
            nc.scalar.dma_start(wsc[:], wsc_d[:])

            # F[p, b, i] = onehot / mask features for row m = 128b + p
            tge = fpool.tile([128, N_BLK, NF], F32, tag="tge")
            tle = fpool.tile([128, N_BLK, NF], F32, tag="tle")
            fall = fpool.tile([128, N_BLK, NF], F32R, tag="fall")
            lms_b = lms_t[:, :, None].broadcast_to([128, N_BLK, 10])
            cnt_b = cnt_t[:, :, None].broadcast_to([128, N_BLK, 12])
            nc.vector.tensor_tensor(
                tge[:, :, 0:10], lms_b,
                lo_t[:, None, 0:10].broadcast_to([128, N_BLK, 10]), ge)
            nc.vector.tensor_tensor(
                tge[:, :, 10:NF], cnt_b,
                lo_t[:, None, 10:NF].broadcast_to([128, N_BLK, 12]), ge)
            nc.vector.tensor_tensor(
                tle[:, :, 0:10], lms_b,
                hi_t[:, None, 0:10].broadcast_to([128, N_BLK, 10]), le)
            nc.vector.tensor_tensor(
                tle[:, :, 10:NF], cnt_b,
                hi_t[:, None, 10:NF].broadcast_to([128, N_BLK, 12]), le)
            nc.vector.tensor_mul(fall[:], tge[:], tle[:])

            osb_tiles = {}
            pending = None

            def emit_score(g, ht):
                sc = pss.tile([1, GROUP], F32, tag="pss")
                nc.tensor.matmul(sc[:], wsc[:], ht[:], start=True, stop=True)
                sq = g // SG
                if g % SG == 0:
                    osb_t = posb.tile([1, SG * GROUP], F32, tag="osb")
                    osb_tiles[sq] = osb_t
                orow = osb_tiles[sq][0:1, GROUP * (g % SG):GROUP * (g % SG + 1)]
                if g % 2 == 0:
                    nc.vector.tensor_copy(orow, sc[:])
                else:
                    nc.scalar.copy(orow, sc[:])
                if g % SG == SG - 1:
                    nc.gpsimd.dma_start(out_r[sq:sq + 1, :],
                                        osb_tiles.pop(sq)[:])

            def load_sg(s):
                xts = []
                for kp in range(KCH // 2):
                    xk = px.tile([128, 2, SG * GROUP], F32R, tag="xin")
                    if s == 0:
                        # split so group 0's chunks land first
                        nc.sync.dma_start(xk[:, :, 0:GROUP],
                                          xt_r[:, kp, :, s, 0:GROUP])
                        nc.sync.dma_start(xk[:, :, GROUP:],
                                          xt_r[:, kp, :, s, GROUP:])
                    else:
                        nc.sync.dma_start(xk[:], xt_r[:, kp, :, s, :])
                    xts.append(xk)
                return xts

            sg_tiles = {0: load_sg(0), 1: load_sg(1)}
            for s in range(N_SG):
                if s + 2 < N_SG:
                    sg_tiles[s + 2] = load_sg(s + 2)
                xts = sg_tiles.pop(s)
                for gi in range(SG):
                    g = SG * s + gi
                    off = GROUP * gi
                    if pending is not None:
                        emit_score(*pending)

                    zt = psz.tile([H, GROUP], F32, tag="psz")
                    for k in range(KCH):
                        nc.tensor.matmul(zt[:], w1t[:, k, :],
                                         xts[k // 2][:, k % 2,
                                                     off:off + GROUP],
                                         start=(k == 0), stop=False)

                    # transpose the 4 feature blocks of this group
                    psft = psf.tile([NF, GROUP], F32R, tag="psf")
                    for j in range(4):
                        b = 4 * g + j
                        nc.tensor.transpose(
                            psft[:, 128 * j:128 * (j + 1)],
                            fall[:, b, :], ident)
                    # ht rows 0..63 = relu(z.T), rows 64..85 = F.T
                    ht = pht.tile([H + NF, GROUP], F32R, tag="ht")
                    if g % 2 == 0:
                        nc.vector.tensor_copy(ht[H:H + NF, :], psft[:])
                    else:
                        nc.scalar.copy(ht[H:H + NF, :], psft[:])

                    nc.tensor.matmul(zt[:], tcat, ht[H:H + NF, :],
                                     start=False, stop=True)

                    nc.scalar.activation(ht[0:H, :], zt[:], relu)
                    pending = (g, ht)
                if s == N_SG - 1:
                    emit_score(*pending)
                    pending = None

    nc.compile()
    _CACHE["nc"] = nc
    return nc


def _prepare_maps(ment_emb, mem_vectors, dist_table, counter_table,
                  W1, b1, W2, b2, ent_counter, last_mention_start, ment_start):
    f32 = np.float32
    ment = np.asarray(ment_emb, f32)
    mem = np.asarray(mem_vectors, f32)
    W1 = np.asarray(W1, f32)
    ms = float(np.asarray(ment_start).astype(np.float64))

    W1m, W1r, W1h = W1[0:D], W1[D:2 * D], W1[2 * D:3 * D]
    W1d, W1c = W1[3 * D:3 * D + E], W1[3 * D + E:3 * D + 2 * E]

    w1eff = (W1m + ment[:, None] * W1h).astype(f32)              # [768, 64]
    bias_vec = (np.asarray(b1, f32) + ment @ W1r).astype(f32)    # [64]
    T_d = (np.asarray(dist_table, f32) @ W1d + bias_vec).astype(f32)
    T_c = (np.asarray(counter_table, f32) @ W1c).astype(f32)
    b2v = float(np.asarray(b2, f32).reshape(-1)[0])

    tcat = np.concatenate(
        [T_d, T_c, np.full((1, H), -BIG, f32), np.zeros((1, H), f32)], 0)
    # single score matmul: rows 0..63 act on relu(z.T), rows 64..85 on F.T
    wsc = np.zeros((H + NF, 1), f32)
    wsc[0:H, 0] = np.asarray(W2, f32).reshape(-1)
    wsc[H + 20, 0] = -10000.0 - b2v
    wsc[H + 21, 0] = b2v

    # bucket i covers c in [A[i], B[i]] (identity below 5, log2 above, clip 9)
    A = np.array([-1e9, 1, 2, 3, 4, 5, 8, 16, 32, 64], np.float64)
    B = np.array([0, 1, 2, 3, 4, 7, 15, 31, 63, 1e9], np.float64)
    # dist bucket in lms terms: dist = ms - lms in [A,B] <=> lms in [ms-B, ms-A]
    lo = np.concatenate([ms - B, A, [-1e9], [-1e9]]).astype(f32)
    hi = np.concatenate([ms - A, B, [0.0], [1e9]]).astype(f32)
    lo_rep = np.ascontiguousarray(np.broadcast_to(lo, (128, NF)))
    hi_rep = np.ascontiguousarray(np.broadcast_to(hi, (128, NF)))

    lms_f = np.asarray(last_mention_start).astype(f32)
    cnt_f = np.asarray(ent_counter).astype(f32)

    in_maps = []
    for c in range(N_CORES):
        sl = slice(c * MS, (c + 1) * MS)
        in_maps.append(dict(
            xt=np.ascontiguousarray(mem[sl].T),
            lms=np.ascontiguousarray(lms_f[sl].reshape(N_BLK, 128).T),
            cnt=np.ascontiguousarray(cnt_f[sl].reshape(N_BLK, 128).T),
            w1=w1eff, tcat=tcat, wsc=wsc, lo=lo_rep, hi=hi_rep))
    return in_maps


def _postprocess(results):
    out = np.empty(M + 1, np.float32)
    for c in range(N_CORES):
        out[c * MS:(c + 1) * MS] = results[c]["out"]
    out[M] = 0.0
    return out


def run_spmd(in_maps, trace=False):
    from concourse.bass_utils import run_bass_kernel_spmd
    nc = _build()
    return run_bass_kernel_spmd(nc, in_maps, list(range(N_CORES)), trace=trace)


def kernel(**inputs):
    in_maps = _prepare_maps(**inputs)
    res = run_spmd(in_maps, trace=False)
    return _postprocess(res.results)



# revision 4
# speedup vs baseline: 1.1369x; 1.1369x over previous
"""Trainium2 Bass kernel for the BaseMemory coref scoring module.

Computes, for full inputs (M=65536 memory slots, D=768, E=20, H=64):
    score = relu(pair @ W1 + b1) @ W2 + b2, masked with ent_counter>0,
    where pair = [mem, ment, mem*ment, dist_emb, cnt_emb].

Sharding: data-parallel over the cluster dimension M across 8 NeuronCores.

The kernel is memory-bound on streaming mem_vectors, so everything that can
be folded away is folded away on the host (exact f64 algebra, O(M*D) data
prep only — all O(M*D*H) FLOPs stay on device):

  - mem@W1_mem + (mem*ment)@W1_had = mem @ (W1_mem + diag(ment)@W1_had) =: mem @ W
  - the bucketized feature-table + bias contribution t[m] (a 64-vector from a
    100-entry table) is folded INTO the mem vectors: with A = W (W^T W)^-1,
    W^T (mem + A t) = W^T mem + t exactly.  So the device computes just
        score = W2^T relu(W^T x')   with x' = mem + A t[idx]
  - bucket indices are computed integer-exactly on host (frexp, no float log)
  - the ent_counter<=0 mask (-10000) and the +b2 offset are applied on host
    after gathering the per-core outputs.

x' streams as bf16 (12.6 MiB/core instead of 24 MiB f32), halving the HBM
traffic that bounds the kernel; scores accumulate in fp32 PSUM, keeping the
max relative error ~1e-3 (tolerance 2e-2).
"""

import os
import numpy as np

# The bass kernel executes through the axon PJRT backend; make sure jax can
# see it even if the caller pinned JAX_PLATFORMS (e.g. to "cpu").
_jp = os.environ.get("JAX_PLATFORMS")
if _jp is not None and _jp != "" and "axon" not in _jp:
    os.environ["JAX_PLATFORMS"] = "axon," + _jp

M, D, E, H = 65536, 768, 20, 64
N_CORES = 8
MS = M // N_CORES          # rows per core = 8192
GROUP = 512                # rows per PE matmul group
NG = MS // GROUP           # 16 groups per core
KCH = D // 128             # 6 contraction chunks
PAIR_DMA = 2               # groups per x DMA

_CACHE = {}


def _build():
    """Build + compile the 8-core SPMD bass program once per process."""
    if "nc" in _CACHE:
        return _CACHE["nc"]

    import concourse.bass as bass
    import concourse.mybir as mybir
    import concourse.tile as tile
    from concourse import bacc

    F32 = mybir.dt.float32
    F32R = mybir.dt.float32r
    BF16 = mybir.dt.bfloat16

    nc = bacc.Bacc("TRN2", target_bir_lowering=False, debug=False,
                   enable_asserts=False, num_devices=N_CORES)

    xq_d = nc.dram_tensor("xq", [128, NG * KCH * GROUP], BF16,
                          kind="ExternalInput").ap()
    w1_d = nc.dram_tensor("w1", [D, H], BF16, kind="ExternalInput").ap()
    wsc_d = nc.dram_tensor("wsc", [H, 1], F32R, kind="ExternalInput").ap()
    out_d = nc.dram_tensor("out", [MS], F32, kind="ExternalOutput").ap()

    # host layout: xq[p, g, k, c] = x'[128k + p, 512g + c] for this core
    xq_r = xq_d.rearrange("p (g x) -> p g x", g=NG)     # [128, 16, 3072]
    w1_r = w1_d.rearrange("(k p) n -> p k n", p=128)    # [128, 6, 64]
    out_r = out_d.rearrange("(q c) -> q c", q=NG // 4)  # [4, 2048]

    relu = mybir.ActivationFunctionType.Relu

    with tile.TileContext(nc) as tc:
        with (
            tc.tile_pool(name="consts", bufs=1) as cpool,
            tc.tile_pool(name="xin", bufs=5) as px,
            tc.tile_pool(name="ht", bufs=4) as pht,
            tc.tile_pool(name="osb", bufs=2) as posb,
            tc.tile_pool(name="psz", bufs=4, space="PSUM") as psz,
            tc.tile_pool(name="pss", bufs=2, space="PSUM") as pss,
        ):
            # consts go on the scalar HWDGE queue so the big xq DMAs
            # (sync queue) start immediately
            w1t = cpool.tile([128, KCH, H], BF16, tag="w1t")
            nc.scalar.dma_start(w1t[:], w1_r[:])
            wsc = cpool.tile([H, 1], F32R, tag="wsc")
            nc.scalar.dma_start(wsc[:], wsc_d[:])

            def load_pair(s):
                xk = px.tile([128, PAIR_DMA, KCH * GROUP], BF16, tag="xin")
                nc.sync.dma_start(xk[:], xq_r[:, PAIR_DMA * s:
                                              PAIR_DMA * (s + 1), :])
                return xk

            n_pairs = NG // PAIR_DMA
            tiles = {s: load_pair(s) for s in range(3)}
            osb_tiles = {}
            pending = None

            for g in range(NG):
                s = g // PAIR_DMA
                if g % PAIR_DMA == 0 and s + 3 < n_pairs:
                    tiles[s + 3] = load_pair(s + 3)
                xk = tiles[s]

                zt = psz.tile([H, GROUP], F32, tag="psz")
                for k in range(KCH):
                    nc.tensor.matmul(zt[:], w1t[:, k, :],
                                     xk[:, g % PAIR_DMA,
                                        k * GROUP:(k + 1) * GROUP],
                                     start=(k == 0), stop=(k == KCH - 1))

                # score matmul for the previous group goes to the PE AFTER
                # this group's z matmuls, so its relu has time to finish and
                # the PE never stalls on the Scalar engine.
                if pending is not None:
                    gp, htp = pending
                    sc = pss.tile([1, GROUP], F32, tag="pss")
                    nc.tensor.matmul(sc[:], wsc[:], htp[:],
                                     start=True, stop=True)
                    q = gp // 4
                    if gp % 4 == 0:
                        osb_t = posb.tile([1, 4 * GROUP], F32, tag="osb")
                        osb_tiles[q] = osb_t
                    orow = osb_tiles[q][0:1, GROUP * (gp % 4):
                                        GROUP * (gp % 4 + 1)]
                    if gp % 2 == 0:
                        nc.vector.tensor_copy(orow, sc[:])
                    else:
                        nc.scalar.copy(orow, sc[:])
                    if gp % 4 == 3:
                        nc.scalar.dma_start(out_r[q:q + 1, :],
                                            osb_tiles.pop(q)[:])

                ht = pht.tile([H, GROUP], F32R, tag="ht")
                nc.scalar.activation(ht[:], zt[:], relu)
                pending = (g, ht)

            # drain the last group
            gp, htp = pending
            sc = pss.tile([1, GROUP], F32, tag="pss")
            nc.tensor.matmul(sc[:], wsc[:], htp[:], start=True, stop=True)
            q = gp // 4
            orow = osb_tiles[q][0:1, GROUP * (gp % 4):GROUP * (gp % 4 + 1)]
            nc.vector.tensor_copy(orow, sc[:])
            nc.scalar.dma_start(out_r[q:q + 1, :], osb_tiles.pop(q)[:])

    nc.compile()
    _CACHE["nc"] = nc
    return nc


def _bucket(c):
    """Integer-exact replica of the reference's get_bucket (identity <=4,
    floor(log2)+3 above, clipped to [0, 9])."""
    c = np.asarray(c, np.int64)
    cpos = np.maximum(c, 1).astype(np.float64)
    lg = np.frexp(cpos)[1] - 1          # exact floor(log2) for integers
    idx = np.where(c <= 4, c, lg + 3)
    return np.clip(idx, 0, 9).astype(np.int64)


def _prepare_maps(ment_emb, mem_vectors, dist_table, counter_table,
                  W1, b1, W2, b2, ent_counter, last_mention_start,
                  ment_start):
    import ml_dtypes

    f64 = np.float64
    ment = np.asarray(ment_emb, f64)
    W1 = np.asarray(W1, f64)
    W1m, W1r, W1h = W1[0:D], W1[D:2 * D], W1[2 * D:3 * D]
    W1d, W1c = W1[3 * D:3 * D + E], W1[3 * D + E:3 * D + 2 * E]

    w1eff = W1m + ment[:, None] * W1h                       # [768, 64]
    bias_vec = np.asarray(b1, f64) + ment @ W1r             # [64]
    T_d = np.asarray(dist_table, f64) @ W1d                 # [10, 64]
    T_c = np.asarray(counter_table, f64) @ W1c              # [10, 64]
    # t(bd, bc) = T_d[bd] + T_c[bc] + bias_vec, for all 100 bucket combos
    T_comb = (T_d[:, None, :] + T_c[None, :, :] +
              bias_vec).reshape(100, H)                     # [100, 64]

    # delta[r] solves w1eff^T delta = T_comb[r] (min-norm): the feature/bias
    # contribution is folded into the mem vectors themselves.
    G = w1eff.T @ w1eff
    G += np.eye(H) * (1e-12 * np.trace(G) / H)              # ridge, paranoia
    delta_table = (np.linalg.solve(G, T_comb.T).T @ w1eff.T)  # [100, 768]

    cnt = np.asarray(ent_counter, np.int64)
    dist = int(np.asarray(ment_start)) - np.asarray(last_mention_start,
                                                    np.int64)
    idx = _bucket(dist) * 10 + _bucket(cnt)                 # [M]

    xprime = np.asarray(mem_vectors, np.float32)
    xprime = xprime + delta_table.astype(np.float32)[idx]   # [M, 768]

    w1b = w1eff.astype(ml_dtypes.bfloat16)                  # [768, 64]
    wsc = np.asarray(W2, np.float32).reshape(H, 1)

    in_maps = []
    for c in range(N_CORES):
        xc = xprime[c * MS:(c + 1) * MS]                    # [8192, 768]
        # -> [p, g, k, c] with d = 128k+p, m = 512g+c
        xt = xc.T.reshape(KCH, 128, NG, GROUP).transpose(1, 2, 0, 3)
        xq = np.ascontiguousarray(xt).astype(ml_dtypes.bfloat16)
        in_maps.append(dict(xq=xq.reshape(128, NG * KCH * GROUP),
                            w1=w1b, wsc=wsc))

    b2v = float(np.asarray(b2, np.float64).reshape(-1)[0])
    return in_maps, (cnt <= 0), b2v


def _postprocess(results, masked, b2v):
    out = np.empty(M + 1, np.float32)
    for c in range(N_CORES):
        out[c * MS:(c + 1) * MS] = results[c]["out"]
    if b2v != 0.0:
        out[:M] += np.float32(b2v)
    out[:M][masked] = -10000.0
    out[M] = 0.0
    return out


def run_spmd(in_maps, trace=False):
    from concourse.bass_utils import run_bass_kernel_spmd
    nc = _build()
    return run_bass_kernel_spmd(nc, in_maps, list(range(N_CORES)),
                                trace=trace)


def kernel(**inputs):
    in_maps, masked, b2v = _prepare_maps(**inputs)
    res = run_spmd(in_maps, trace=False)
    return _postprocess(res.results, masked, b2v)


# revision 9
# speedup vs baseline: 1.6075x; 1.4139x over previous
"""Trainium2 Bass kernel for the BaseMemory coref scoring module.

Computes, for full inputs (M=65536 memory slots, D=768, E=20, H=64):
    score = relu(pair @ W1 + b1) @ W2 + b2, masked with ent_counter>0,
    where pair = [mem, ment, mem*ment, dist_emb, cnt_emb].

Sharding: data-parallel over the cluster dimension M across 8 NeuronCores.

The kernel is memory-bound on streaming mem_vectors, so everything that can
be folded away is folded away on the host (exact f64 algebra, O(M*D) data
prep only — all O(M*D*H) FLOPs stay on device):

  - mem@W1_mem + (mem*ment)@W1_had = mem @ (W1_mem + diag(ment)@W1_had) =: mem @ W
  - the bucketized feature-table + bias contribution t[m] (a 64-vector from a
    100-entry table) is folded INTO the mem vectors: with A = W (W^T W)^-1,
    W^T (mem + A t) = W^T mem + t exactly.  So the device computes just
        score = W2^T relu(W^T x')   with x' = mem + A t[idx]
  - bucket indices are computed integer-exactly on host (frexp, no float log)
  - the ent_counter<=0 mask (-10000) and the +b2 offset are applied on host
    after gathering the per-core outputs.

x' streams as bf16 (12.6 MiB/core instead of 24 MiB f32), halving the HBM
traffic that bounds the kernel; scores accumulate in fp32 PSUM, keeping the
max relative error ~1e-3 (tolerance 2e-2).
"""

import os
import numpy as np

# The bass kernel executes through the axon PJRT backend; make sure jax can
# see it even if the caller pinned JAX_PLATFORMS (e.g. to "cpu").
_jp = os.environ.get("JAX_PLATFORMS")
if _jp is not None and _jp != "" and "axon" not in _jp:
    os.environ["JAX_PLATFORMS"] = "axon," + _jp

M, D, E, H = 65536, 768, 20, 64
N_CORES = 8
MS = M // N_CORES          # rows per core = 8192
GROUP = 512                # rows per PE matmul group
NG = MS // GROUP           # 16 groups per core
KCH = D // 128             # 6 contraction chunks
PAIR_DMA = 2               # groups per x DMA

_CACHE = {}


def _build():
    """Build + compile the 8-core SPMD bass program once per process."""
    if "nc" in _CACHE:
        return _CACHE["nc"]

    import concourse.bass as bass
    import concourse.mybir as mybir
    import concourse.tile as tile
    from concourse import bacc

    F32 = mybir.dt.float32
    F32R = mybir.dt.float32r
    BF16 = mybir.dt.bfloat16

    nc = bacc.Bacc("TRN2", target_bir_lowering=False, debug=False,
                   enable_asserts=False, num_devices=N_CORES)

    xq_d = nc.dram_tensor("xq", [128, NG * KCH * GROUP], BF16,
                          kind="ExternalInput").ap()
    # host pre-swizzles w1 to [p, k, n] so this is ONE contiguous DMA
    # (a "(k p) n" rearrange would emit 768 x 128B descriptors that take
    # ~28us to drain and starve the xq stream)
    w1_d = nc.dram_tensor("w1", [128, KCH * H], BF16,
                          kind="ExternalInput").ap()
    wsc_d = nc.dram_tensor("wsc", [H, 1], F32R, kind="ExternalInput").ap()
    out_d = nc.dram_tensor("out", [MS], F32, kind="ExternalOutput").ap()

    # host layout: xq[p, g, k, c] = x'[128k + p, 512g + c] for this core
    xq_r = xq_d.rearrange("p (g x) -> p g x", g=NG)     # [128, 16, 3072]
    w1_r = w1_d.rearrange("p (k n) -> p k n", k=KCH)    # [128, 6, 64]
    out_r = out_d.rearrange("(q c) -> q c", q=NG // 4)  # [4, 2048]

    relu = mybir.ActivationFunctionType.Relu

    with tile.TileContext(nc) as tc:
        with (
            tc.tile_pool(name="consts", bufs=1) as cpool,
            tc.tile_pool(name="xin", bufs=5) as px,
            tc.tile_pool(name="ht", bufs=4) as pht,
            tc.tile_pool(name="osb", bufs=2) as posb,
            tc.tile_pool(name="psz", bufs=4, space="PSUM") as psz,
            tc.tile_pool(name="pss", bufs=2, space="PSUM") as pss,
        ):
            # consts go on the scalar HWDGE queue so the big xq DMAs
            # (sync queue) start immediately
            w1t = cpool.tile([128, KCH, H], BF16, tag="w1t")
            nc.scalar.dma_start(w1t[:], w1_r[:])
            wsc = cpool.tile([H, 1], F32R, tag="wsc")
            nc.scalar.dma_start(wsc[:], wsc_d[:])

            # DMA chunks in groups: pairs up front (big transfers), the last
            # two groups as singles so the tail compute starts sooner.
            chunk_of = {}          # group -> (chunk_id, idx within chunk)
            chunk_groups = []      # chunk_id -> (first_group, n_groups)
            g0 = 0
            while g0 < NG:
                n = PAIR_DMA if g0 < NG - 2 else 1
                for j in range(n):
                    chunk_of[g0 + j] = (len(chunk_groups), j)
                chunk_groups.append((g0, n))
                g0 += n

            def load_chunk(s):
                first, n = chunk_groups[s]
                xk = px.tile([128, n, KCH * GROUP], BF16, tag="xin")
                nc.sync.dma_start(xk[:], xq_r[:, first:first + n, :])
                return xk

            tiles = {s: load_chunk(s) for s in range(3)}
            osb_tiles = {}
            pending = None

            for g in range(NG):
                s, j = chunk_of[g]
                if j == 0 and s + 3 < len(chunk_groups):
                    tiles[s + 3] = load_chunk(s + 3)
                xk = tiles[s]

                zt = psz.tile([H, GROUP], F32, tag="psz")
                for k in range(KCH):
                    nc.tensor.matmul(zt[:], w1t[:, k, :],
                                     xk[:, j, k * GROUP:(k + 1) * GROUP],
                                     start=(k == 0), stop=(k == KCH - 1))

                # score matmul for the previous group goes to the PE AFTER
                # this group's z matmuls, so its relu has time to finish and
                # the PE never stalls on the Scalar engine.
                if pending is not None:
                    gp, htp = pending
                    sc = pss.tile([1, GROUP], F32, tag="pss")
                    nc.tensor.matmul(sc[:], wsc[:], htp[:],
                                     start=True, stop=True)
                    q = gp // 4
                    if gp % 4 == 0:
                        osb_t = posb.tile([1, 4 * GROUP], F32, tag="osb")
                        osb_tiles[q] = osb_t
                    orow = osb_tiles[q][0:1, GROUP * (gp % 4):
                                        GROUP * (gp % 4 + 1)]
                    if gp % 2 == 0:
                        nc.vector.tensor_copy(orow, sc[:])
                    else:
                        nc.scalar.copy(orow, sc[:])
                    if gp % 4 == 3:
                        nc.scalar.dma_start(out_r[q:q + 1, :],
                                            osb_tiles.pop(q)[:])

                ht = pht.tile([H, GROUP], F32R, tag="ht")
                nc.scalar.activation(ht[:], zt[:], relu)
                pending = (g, ht)

            # drain the last group
            gp, htp = pending
            sc = pss.tile([1, GROUP], F32, tag="pss")
            nc.tensor.matmul(sc[:], wsc[:], htp[:], start=True, stop=True)
            q = gp // 4
            orow = osb_tiles[q][0:1, GROUP * (gp % 4):GROUP * (gp % 4 + 1)]
            nc.vector.tensor_copy(orow, sc[:])
            nc.scalar.dma_start(out_r[q:q + 1, :], osb_tiles.pop(q)[:])

    nc.compile()
    _CACHE["nc"] = nc
    return nc


def _bucket(c):
    """Integer-exact replica of the reference's get_bucket (identity <=4,
    floor(log2)+3 above, clipped to [0, 9])."""
    c = np.asarray(c, np.int64)
    cpos = np.maximum(c, 1).astype(np.float64)
    lg = np.frexp(cpos)[1] - 1          # exact floor(log2) for integers
    idx = np.where(c <= 4, c, lg + 3)
    return np.clip(idx, 0, 9).astype(np.int64)


def _prepare_maps(ment_emb, mem_vectors, dist_table, counter_table,
                  W1, b1, W2, b2, ent_counter, last_mention_start,
                  ment_start):
    import ml_dtypes

    f64 = np.float64
    ment = np.asarray(ment_emb, f64)
    W1 = np.asarray(W1, f64)
    W1m, W1r, W1h = W1[0:D], W1[D:2 * D], W1[2 * D:3 * D]
    W1d, W1c = W1[3 * D:3 * D + E], W1[3 * D + E:3 * D + 2 * E]

    w1eff = W1m + ment[:, None] * W1h                       # [768, 64]
    bias_vec = np.asarray(b1, f64) + ment @ W1r             # [64]
    T_d = np.asarray(dist_table, f64) @ W1d                 # [10, 64]
    T_c = np.asarray(counter_table, f64) @ W1c              # [10, 64]
    # t(bd, bc) = T_d[bd] + T_c[bc] + bias_vec, for all 100 bucket combos
    T_comb = (T_d[:, None, :] + T_c[None, :, :] +
              bias_vec).reshape(100, H)                     # [100, 64]

    # delta[r] solves w1eff^T delta = T_comb[r] (min-norm): the feature/bias
    # contribution is folded into the mem vectors themselves.
    G = w1eff.T @ w1eff
    G += np.eye(H) * (1e-12 * np.trace(G) / H)              # ridge, paranoia
    delta_table = (np.linalg.solve(G, T_comb.T).T @ w1eff.T)  # [100, 768]

    cnt = np.asarray(ent_counter, np.int64)
    dist = int(np.asarray(ment_start)) - np.asarray(last_mention_start,
                                                    np.int64)
    idx = _bucket(dist) * 10 + _bucket(cnt)                 # [M]

    xprime = np.asarray(mem_vectors, np.float32)
    xprime = xprime + delta_table.astype(np.float32)[idx]   # [M, 768]

    # [p, k, n] swizzle (d = 128k + p) -> one contiguous DMA on device
    w1b = np.ascontiguousarray(
        w1eff.reshape(KCH, 128, H).transpose(1, 0, 2)
    ).astype(ml_dtypes.bfloat16).reshape(128, KCH * H)
    wsc = np.asarray(W2, np.float32).reshape(H, 1)

    in_maps = []
    for c in range(N_CORES):
        xc = xprime[c * MS:(c + 1) * MS]                    # [8192, 768]
        # -> [p, g, k, c] with d = 128k+p, m = 512g+c
        xt = xc.T.reshape(KCH, 128, NG, GROUP).transpose(1, 2, 0, 3)
        xq = np.ascontiguousarray(xt).astype(ml_dtypes.bfloat16)
        in_maps.append(dict(xq=xq.reshape(128, NG * KCH * GROUP),
                            w1=w1b, wsc=wsc))

    b2v = float(np.asarray(b2, np.float64).reshape(-1)[0])
    return in_maps, (cnt <= 0), b2v


def _postprocess(results, masked, b2v):
    out = np.empty(M + 1, np.float32)
    for c in range(N_CORES):
        out[c * MS:(c + 1) * MS] = results[c]["out"]
    if b2v != 0.0:
        out[:M] += np.float32(b2v)
    out[:M][masked] = -10000.0
    out[M] = 0.0
    return out


def run_spmd(in_maps, trace=False):
    from concourse.bass_utils import run_bass_kernel_spmd
    nc = _build()
    return run_bass_kernel_spmd(nc, in_maps, list(range(N_CORES)),
                                trace=trace)


def kernel(**inputs):
    in_maps, masked, b2v = _prepare_maps(**inputs)
    res = run_spmd(in_maps, trace=False)
    return _postprocess(res.results, masked, b2v)
